# revision 1
# baseline (speedup 1.0000x reference)
"""Trainium2 Bass kernel for single-head causal attention.

Transposed-softmax layout (S^T, no PE transposes, no max-subtraction),
interleaved-key causal skip, host flash-combine, plus host-side weight
fusion:

    Mh  = WQ @ WK^T / sqrt(d)        (host, activation-independent)
    S^T = K~^T^T-contraction where K~^T = Mh @ Xk^T

Each core computes K~^T = Mh @ Xk^T  [din, k-half] for its own key-half
directly from the raw key inputs — this replaces BOTH the K projection
and the old KQ^T = WQ @ K^T phase (one 1024x1024x1024 matmul instead of
two).  The score matmuls then contract raw Xq^T streamed from DRAM.

Per-core phases:  PE warmup (bridges the DMA cold start so the DVFS
ramp completes early) -> K~ -> g3 score block (the largest attention
group's scores need only kqt + Xq^T, so they run here while the V-phase
inputs stream) -> V proj -> attention (remaining groups, largest first).
Outputs: ohat [2048, 1024] bf16 (unnormalized), l [1, 2048]; the host
combines out = (ohat0 + ohat1) / (l0 + l1).
"""

import numpy as np

import concourse.bass as bass
from concourse import bacc
import concourse.mybir as mybir
import concourse.tile as tile
from concourse.bass_utils import run_bass_kernel_spmd

P = 128
B, S, DIN, DOUT = 4, 2048, 1024, 1024
KSH = S // 2        # key rows per core
KO = DIN // P       # 8 contraction sub-tiles for the projections
DO = DOUT // P      # 8 dout sub-tiles
NT = KSH // P       # 8 key tiles per core
QG = 512            # query-group width (psum free dim)
NG = S // QG        # 4 query groups
F32 = mybir.dt.float32
F32R = mybir.dt.float32r
BF16 = mybir.dt.bfloat16
NEG = -1.0e9

_NC_CACHE = {}


def _load_sliced(nc, dst, src, width, nslice, first_only=False, rest_only=False):
    """DMA a [128, KO, width] tensor in dout-slices for early availability."""
    step = width // nslice
    slices = range(nslice)
    if first_only:
        slices = range(1)
    elif rest_only:
        slices = range(1, nslice)
    for s in slices:
        nc.sync.dma_start(
            out=dst[:, :, s * step : (s + 1) * step],
            in_=src[:, :, s * step : (s + 1) * step],
        )


def _stream_projection(
    nc, pools, x3, w3, consume, seq, chunk, lhs_from_x, after_first_dma=None
):
    xpool, pspool = pools
    for c in range(seq // chunk):
        x_sb = xpool.tile([P, KO, chunk], F32R, name="xstream")
        for o in range(0, KO, 2):
            nc.sync.dma_start(
                out=x_sb[:, o : o + 2, :],
                in_=x3[:, o : o + 2, c * chunk : (c + 1) * chunk],
            )
        if c == 0 and after_first_dma is not None:
            after_first_dma()
        if lhs_from_x:
            for t in range(chunk // P):
                for dh in range(DOUT // 512):
                    ps = pspool.tile([P, 512], F32, name="proj_ps")
                    for k in range(KO):
                        nc.tensor.matmul(
                            ps,
                            lhsT=x_sb[:, k, t * P : (t + 1) * P],
                            rhs=w3[:, k, dh * 512 : (dh + 1) * 512],
                            start=(k == 0),
                            stop=(k == KO - 1),
                        )
                    consume(ps, c * chunk + t * P, dh * 512)
        else:
            for o in range(DO):
                ps = pspool.tile([P, chunk], F32, name="proj_ps")
                for k in range(KO):
                    nc.tensor.matmul(
                        ps,
                        lhsT=w3[:, k, o * P : (o + 1) * P],
                        rhs=x_sb[:, k, :],
                        start=(k == 0),
                        stop=(k == KO - 1),
                    )
                consume(ps, o, c * chunk)



def _emit_score_chunk(nc, psS, epool, m0_sb, kqt, xq_g, slab, zeros_sb, g, kt):
    """Scores+exp for one (group, k-tile) chunk of S^T.  kt == 2g+1's first
    256-query half is fully masked for BOTH cores (interleaved-key
    geometry): zero-fill that slab half and compute only the second half.
    The causal mask pattern is group-independent (k_local + 128*hk >
    q_local within the diagonal chunk), so a single resident [128, 512]
    tile m0_sb serves every chunk; the half chunk uses its first half."""
    H = QG // 2
    if kt == 2 * g + 1:
        ps = psS.tile([P, QG], F32, name="score_ps")
        ph = ps[:, H:]
        for io in range(KO):
            nc.tensor.matmul(
                ph, lhsT=kqt[:, io, kt * P : (kt + 1) * P],
                rhs=xq_g[:, io, H:],
                start=(io == 0), stop=(io == KO - 1),
            )
        nc.vector.tensor_copy(slab[:, kt, :H], zeros_sb)
        # exp on f32 psum first, then zero the masked slots with a 0/1
        # multiply (post-exp, so bf16 rounding never touches raw logits)
        et = epool.tile([P, H], F32, name="exp_tmp")
        nc.scalar.activation(et, ph, mybir.ActivationFunctionType.Exp)
        nc.vector.tensor_tensor(
            slab[:, kt, H:], et, m0_sb[:, :H], mybir.AluOpType.mult
        )
        return
    ps = psS.tile([P, QG], F32, name="score_ps")
    for io in range(KO):
        nc.tensor.matmul(
            ps, lhsT=kqt[:, io, kt * P : (kt + 1) * P], rhs=xq_g[:, io, :],
            start=(io == 0), stop=(io == KO - 1),
        )
    if kt >= 2 * g:
        et = epool.tile([P, QG], F32, name="exp_tmp")
        nc.scalar.activation(et, ps, mybir.ActivationFunctionType.Exp)
        nc.vector.tensor_tensor(
            slab[:, kt, :], et, m0_sb, mybir.AluOpType.mult
        )
    else:
        nc.scalar.activation(
            slab[:, kt, :], ps, mybir.ActivationFunctionType.Exp
        )

def _build_bass():
    nc = bacc.Bacc()
    xqT = nc.declare_dram_parameter("xqT", [DIN, S], F32R, isOutput=False)
    xkT = nc.declare_dram_parameter("xkT", [DIN, KSH], F32R, isOutput=False)
    xvT = nc.declare_dram_parameter("xvT", [DIN, KSH], F32R, isOutput=False)
    mhT = nc.declare_dram_parameter("mhT", [P, KO * KO * P], F32R, isOutput=False)
    wv = nc.declare_dram_parameter("wv", [P, KO * KO * P], F32R, isOutput=False)
    maskT = nc.declare_dram_parameter("maskT", [P, QG], F32, isOutput=False)
    ohat = nc.declare_dram_parameter("ohat", [S, DOUT], BF16, isOutput=True)
    l_out = nc.declare_dram_parameter("l", [1, S], F32, isOutput=True)

    xq3 = xqT[:, :].rearrange("(o p) q -> p o q", p=P)
    xk3 = xkT[:, :].rearrange("(o p) s -> p o s", p=P)
    xv3 = xvT[:, :].rearrange("(o p) s -> p o s", p=P)
    # host pre-swizzled so each output-slice load is one contiguous 4KB
    # per-partition line: mh4[p, it, k, ii] = Mh.T[k*128+p, it*128+ii]
    mh4 = mhT[:, :].rearrange("p (t k x) -> p t k x", t=KO, k=KO)
    wv4 = wv[:, :].rearrange("p (s k x) -> p s k x", s=KO, k=KO)

    with tile.TileContext(nc) as tc:
        # PE warmup: ~36 dummy matmuls bridge the DMA cold start so the
        # tensor engine's frequency ramp completes before real work arrives
        with (
            tc.tile_pool(name="warm", bufs=1) as warmpool,
            tc.tile_pool(name="ps_w", bufs=1, space="PSUM") as pswarm,
        ):
            wsc_f = warmpool.tile([P, 512], F32, name="warm_f32")
            nc.vector.memset(wsc_f, 0.0)
            wsc = warmpool.tile([P, 512], F32R, name="warm_sc")
            nc.vector.tensor_copy(wsc, wsc_f)
            wps = pswarm.tile([P, 512], F32, name="warm_ps")
            for i in range(36):
                nc.tensor.matmul(
                    wps, lhsT=wsc[:, 0:P], rhs=wsc,
                    start=(i == 0), stop=(i == 35),
                )
        with tc.tile_pool(name="persist", bufs=1) as persist:
            v_sb = persist.tile([P, NT, DOUT], F32R, name="v")     # V   [k, dout]
            kqt_sb = persist.tile([P, KO, KSH], F32R, name="kqt")  # K~^T [din, k]

            # ---- Phase K~: K~^T = Mh @ Xk^T for this core's key blocks
            with (
                tc.tile_pool(name="wpool_m", bufs=1) as wpool,
                tc.tile_pool(name="xs_k", bufs=2) as xpool,
                tc.tile_pool(name="ps_k", bufs=4, space="PSUM") as pspool,
            ):
                mh_sb = wpool.tile([P, KO, DIN], F32R, name="mh")
                nc.sync.dma_start(
                    out=mh_sb[:, :, 0:P], in_=mh4[:, 0]
                )

                def consume_kq(ps, o, j):
                    nc.vector.tensor_copy(kqt_sb[:, o, j : j + 512], ps)

                def _mh_rest():
                    for s in range(1, KO):
                        nc.sync.dma_start(
                            out=mh_sb[:, :, s * P : (s + 1) * P],
                            in_=mh4[:, s],
                        )

                _stream_projection(
                    nc, (xpool, pspool), xk3, mh_sb, consume_kq,
                    seq=KSH, chunk=512, lhs_from_x=False,
                    after_first_dma=_mh_rest,
                )

            # Prefetch issue order matters: DMA queues fair-share HBM
            # bandwidth, so bulk prefetch issued early starves the critical
            # path.  Order: Xq^T for the largest attention group (needed by
            # the g3 score block right after K~) -> wv first half -> xv
            # chunk 0 -> wv second half.
            xqpool_cm = tc.tile_pool(name="xq_g", bufs=2)
            xqpool = xqpool_cm.__enter__()
            xq_first = xqpool.tile([P, KO, QG], F32R, name="xq_group")
            # half-major so the g3 block's first 256-wide score chunk is
            # gated on 1MB instead of the full 2MB
            for h in range(2):
                for o in range(0, KO, 4):
                    nc.sync.dma_start(
                        out=xq_first[:, o : o + 4, h * 256 : (h + 1) * 256],
                        in_=xq3[
                            :, o : o + 4,
                            (NG - 1) * QG + h * 256 : (NG - 1) * QG
                            + (h + 1) * 256,
                        ],
                    )
            slabf_cm = tc.tile_pool(name="slab_first", bufs=1)
            slabfpool = slabf_cm.__enter__()
            slab_first = slabfpool.tile([P, NT, QG], F32R, name="expT_first")
            zeros_sb = slabfpool.tile([P, QG // 2], F32, name="zeros")
            nc.vector.memset(zeros_sb, 0.0)
            m0_sb = slabfpool.tile([P, QG], F32, name="mask0")
            nc.sync.dma_start(out=m0_sb, in_=maskT[:, :])
            epool_cm = tc.tile_pool(name="exp_tmp", bufs=2)
            epool = epool_cm.__enter__()
            # wv lives above the attention-phase pools in the stack so its
            # 32KB frees after the V projection
            vwpool_cm = tc.tile_pool(name="wpool_v", bufs=1)
            vwpool = vwpool_cm.__enter__()
            wv_sb = vwpool.tile([P, KO, DOUT], F32R, name="wv")
            for s in range(4):
                nc.sync.dma_start(
                    out=wv_sb[:, :, s * 128 : (s + 1) * 128], in_=wv4[:, s]
                )
            vxpool_cm = tc.tile_pool(name="xs_v", bufs=2)
            vxpool = vxpool_cm.__enter__()
            xv_c0 = vxpool.tile([P, KO, 256], F32R, name="xstream")
            for o in range(0, KO, 4):
                nc.sync.dma_start(
                    out=xv_c0[:, o : o + 4, :], in_=xv3[:, o : o + 4, 0:256]
                )

            def g3_scores(kts):
                """Score+exp for the first (largest) query group, interleaved
                between V chunks to cover V's DMA latency."""
                with tc.tile_pool(name="ps_sf", bufs=3, space="PSUM") as psSf:
                    for kt in kts:
                        _emit_score_chunk(
                            nc, psSf, epool, m0_sb, kqt_sb, xq_first,
                            slab_first, zeros_sb, NG - 1, kt,
                        )

            # ---- g3 score block: the largest attention group's scores
            # need only kqt + xq_first, so they run between K~ and the V
            # projection — giving the wv/xv stream an extra ~15us to land
            # and freeing the xq buffer early for the g1/g0 prefetches.
            with tc.tile_pool(name="ps_sf0", bufs=2, space="PSUM") as psH:
                for kt in (0, 1):      # unmasked for g3; 256-wide halves so
                    for h in (0, 1):   # the PE starts on half of xq_first
                        psh = psH.tile([P, QG // 2], F32, name="score_h")
                        for io in range(KO):
                            nc.tensor.matmul(
                                psh,
                                lhsT=kqt_sb[:, io, kt * P : (kt + 1) * P],
                                rhs=xq_first[:, io, h * 256 : (h + 1) * 256],
                                start=(io == 0),
                                stop=(io == KO - 1),
                            )
                        nc.scalar.activation(
                            slab_first[:, kt, h * 256 : (h + 1) * 256], psh,
                            mybir.ActivationFunctionType.Exp,
                        )
            g3_scores(list(range(2, NT)))

            # ---- Phase V: V projection for this core's key blocks
            with (
                tc.tile_pool(name="ps_v", bufs=4, space="PSUM") as pspool,
            ):
                xpool = vxpool
                for c in range(KSH // 256):
                    if c == 0:
                        x_sb = xv_c0
                        # second half of WV (for dh=1)
                        for s in range(4, 8):
                            nc.sync.dma_start(
                                out=wv_sb[:, :, s * 128 : (s + 1) * 128],
                                in_=wv4[:, s],
                            )
                    else:
                        x_sb = xpool.tile([P, KO, 256], F32R, name="xstream")
                        for o in range(0, KO, 4):
                            nc.sync.dma_start(
                                out=x_sb[:, o : o + 4, :],
                                in_=xv3[:, o : o + 4, c * 256 : (c + 1) * 256],
                            )
                    dw = 256 if c == 0 else 512
                    for t in range(2):
                        for dh in range(DOUT // dw):
                            ps = pspool.tile([P, dw], F32, name="proj_ps")
                            for k in range(KO):
                                nc.tensor.matmul(
                                    ps,
                                    lhsT=x_sb[:, k, t * P : (t + 1) * P],
                                    rhs=wv_sb[:, k, dh * dw : (dh + 1) * dw],
                                    start=(k == 0),
                                    stop=(k == KO - 1),
                                )
                            s0 = c * 256 + t * P
                            nc.vector.tensor_copy(
                                v_sb[:, s0 // P, dh * dw : (dh + 1) * dw], ps
                            )
            vxpool_cm.__exit__(None, None, None)
            vwpool_cm.__exit__(None, None, None)

            # ---- Phase A: causal-skip transposed-softmax attention.
            # Raw Xq^T streams from DRAM per query group; groups run
            # largest-first so prefetch covers the small (DMA-bound) ones.
            with (
                tc.tile_pool(name="ones", bufs=1) as onepool,
                tc.tile_pool(name="lrow", bufs=1) as lpool,
                tc.tile_pool(name="slab", bufs=3) as slabpool,
                tc.tile_pool(name="ao", bufs=3) as aopool,
                tc.tile_pool(name="ps_s", bufs=4, space="PSUM") as psS,
                tc.tile_pool(name="ps_l", bufs=1, space="PSUM") as psL,
                tc.tile_pool(name="ps_o", bufs=3, space="PSUM") as psO,
            ):
                ones_f32 = onepool.tile([P, 1], F32, name="ones_f32")
                nc.vector.memset(ones_f32, 1.0)
                ones_sb = onepool.tile([P, 1], F32R, name="ones")
                nc.vector.tensor_copy(ones_sb, ones_f32)
                l_sb = lpool.tile([1, S], F32, name="l_row")

                for g in reversed(range(NG)):
                    lim = min(NT, 2 * g + 2)   # k-tiles actually attended
                    if g == NG - 1:
                        # scores already computed inside the V phase
                        slab = slab_first
                        xq_g = None
                    else:
                        xq_g = xqpool.tile([P, KO, QG], F32R, name="xq_group")
                        for o in range(0, KO, 2):
                            nc.sync.dma_start(
                                out=xq_g[:, o : o + 2, :],
                                in_=xq3[:, o : o + 2, g * QG : (g + 1) * QG],
                            )
                        slab = slabpool.tile([P, NT, QG], F32R, name="expT")
                    for kt in range(lim if g < NG - 1 else 0):
                        _emit_score_chunk(
                            nc, psS, epool, m0_sb, kqt_sb, xq_g,
                            slab, zeros_sb, g, kt,
                        )

                    ps_l = psL.tile([1, QG], F32, name="l_ps")
                    for kt in range(lim):
                        nc.tensor.matmul(
                            ps_l,
                            lhsT=ones_sb,
                            rhs=slab[:, kt, :],
                            start=(kt == 0),
                            stop=(kt == lim - 1),
                        )
                    nc.vector.tensor_copy(l_sb[:, g * QG : (g + 1) * QG], ps_l)

                    for t in range(QG // P):
                        kts = list(range(lim - 1)) if t < 2 else list(range(lim))
                        o_sb = aopool.tile([P, DOUT], BF16, name="attn_out")
                        q0 = g * QG + t * P
                        for dh in range(DOUT // 512):
                            ps = psO.tile([P, 512], F32, name="out_ps")
                            for kt in kts:
                                nc.tensor.matmul(
                                    ps,
                                    lhsT=slab[:, kt, t * P : (t + 1) * P],
                                    rhs=v_sb[:, kt, dh * 512 : (dh + 1) * 512],
                                    start=(kt == kts[0]),
                                    stop=(kt == kts[-1]),
                                )
                            if dh == 0:
                                nc.scalar.copy(
                                    o_sb[:, dh * 512 : (dh + 1) * 512], ps
                                )
                            else:
                                nc.vector.tensor_copy(
                                    o_sb[:, dh * 512 : (dh + 1) * 512], ps
                                )
                            nc.scalar.dma_start(
                                out=ohat[q0 : q0 + P, dh * 512 : (dh + 1) * 512],
                                in_=o_sb[:, dh * 512 : (dh + 1) * 512],
                            )

                nc.sync.dma_start(out=l_out[:, :], in_=l_sb)
            epool_cm.__exit__(None, None, None)
            slabf_cm.__exit__(None, None, None)
            xqpool_cm.__exit__(None, None, None)
    nc.finalize()
    return nc


def _get_nc():
    if "nc" not in _NC_CACHE:
        _NC_CACHE["nc"] = _build_bass()
    return _NC_CACHE["nc"]


def _key_index(hk):
    """Global key rows owned by core hk: interleaved 128-row blocks."""
    blocks = np.arange(hk, S // P, 2)
    return (blocks[:, None] * P + np.arange(P)[None, :]).reshape(-1)


def _mask_tile(hk):
    """Resident multiplicative causal mask for the diagonal score chunk:
    within chunk kt == 2g (global key block 4g+hk), key row k_local masks
    query column q_local iff k_local + 128*hk > q_local; the same
    inequality covers the kt == 2g+1 half chunk on its first 256 columns.
    Applied POST-exp as a 0/1 multiply so raw logits stay f32."""
    k_idx = np.arange(P)[:, None] + P * hk
    q_idx = np.arange(QG)[None, :]
    return np.where(k_idx > q_idx, 0.0, 1.0).astype(np.float32)


def kernel(
    inputs_for_keys,
    inputs_for_values,
    inputs_for_queries,
    WK,
    WV,
    WQ,
    _trace=False,
):
    import ml_dtypes

    xk = np.asarray(inputs_for_keys, dtype=np.float32)
    xv = np.asarray(inputs_for_values, dtype=np.float32)
    xq = np.asarray(inputs_for_queries, dtype=np.float32)
    wv_f = np.asarray(WV, dtype=np.float32)
    wv = np.ascontiguousarray(
        wv_f.reshape(KO, P, KO, P).transpose(1, 2, 0, 3).reshape(P, -1)
    )
    wq = np.asarray(WQ, dtype=np.float32)
    wk = np.asarray(WK, dtype=np.float32)
    # fused score weight: S = Xq (WQ WK^T / sqrt(d)) Xk^T;  mhT = (WQ WK^T).T
    mh_f = ((wk @ wq.T) * np.float32(1.0 / np.sqrt(DOUT))).astype(np.float32)
    # swizzle [j, i] -> [p, it, k, ii]: D[p, it*1024 + k*128 + ii] =
    # mhT[k*128+p, it*128+ii]
    mhT = np.ascontiguousarray(
        mh_f.reshape(KO, P, KO, P).transpose(1, 2, 0, 3).reshape(P, -1)
    )

    masks = {hk: _mask_tile(hk) for hk in (0, 1)}
    kidx = {hk: _key_index(hk) for hk in (0, 1)}
    xqTb = [np.ascontiguousarray(xq[b].T) for b in range(B)]

    in_maps = []
    for i in range(8):
        b, hk = i // 2, i % 2
        in_maps.append(
            {
                "xqT": xqTb[b],
                "xkT": np.ascontiguousarray(xk[b][kidx[hk]].T),
                "xvT": np.ascontiguousarray(xv[b][kidx[hk]].T),
                "mhT": mhT,
                "wv": wv,
                "maskT": masks[hk],
            }
        )

    nc = _get_nc()
    res = run_bass_kernel_spmd(nc, in_maps, list(range(8)), trace=_trace)

    out = np.empty((B, S, DOUT), dtype=np.float32)
    for b in range(B):
        r0 = res.results[2 * b]
        r1 = res.results[2 * b + 1]
        den = (
            np.asarray(r0["l"], np.float32) + np.asarray(r1["l"], np.float32)
        ).reshape(S, 1)
        o01 = np.asarray(r0["ohat"], np.float32) + np.asarray(
            r1["ohat"], np.float32
        )
        out[b] = o01 / den
    if _trace:
        return out, res
    return out



# revision 6
# speedup vs baseline: 1.1225x; 1.1225x over previous
"""Trainium2 Bass kernel for single-head causal attention.

Transposed-softmax layout (S^T, no PE transposes, no max-subtraction),
interleaved-key causal skip, host flash-combine, host-side weight
fusion Mh = WK @ WQ^T / sqrt(d), and a fully-resident fp16/bf16 SBUF
plan:

  score path (Xq, Mh, Xk, K~) in fp16  -- 1.0 PE cycle/row like f32r,
      but half the DMA bytes / SBUF footprint and FWL weight loads
  softmax slab, V, ohat in bf16        -- exp() needs bf16's exponent
      range (scores reach ~+/-50, exp up to ~1e22)
  all matmul accumulation in f32 PSUM; host combines
      out = (ohat0 + ohat1) / (l0 + l1) per batch in f32.

Everything is resident in SBUF (~180 KB/partition of 208), so every
input tile is DMA'd exactly once with no write-after-read hazards:
the PE stream never waits on a buffer-reuse semaphore.  Per-core
phases: warmup (bridges DMA cold start + HAM clock ramp) -> K~ =
Mh @ Xk^T -> g3 score block -> V projection -> attention groups
largest-first.
"""

import numpy as np

import concourse.bass as bass
from concourse import bacc
import concourse.mybir as mybir
import concourse.tile as tile
from concourse.bass_utils import run_bass_kernel_spmd

P = 128
B, S, DIN, DOUT = 4, 2048, 1024, 1024
KSH = S // 2        # key rows per core
KO = DIN // P       # 8 contraction sub-tiles
NT = KSH // P       # 8 key tiles per core
QG = 512            # query-group width (psum free dim)
NG = S // QG        # 4 query groups
F32 = mybir.dt.float32
F16 = mybir.dt.float16
BF16 = mybir.dt.bfloat16
WARM = 10           # warmup matmuls (bridge DMA cold start + HAM ramp)

_NC_CACHE = {}


def _build_bass():
    nc = bacc.Bacc()
    xqT = nc.declare_dram_parameter("xqT", [DIN, S], F16, isOutput=False)
    xkT = nc.declare_dram_parameter("xkT", [DIN, KSH], F16, isOutput=False)
    xvT = nc.declare_dram_parameter("xvT", [DIN, KSH], F16, isOutput=False)
    mhT = nc.declare_dram_parameter("mhT", [P, KO * KO * P], F16, isOutput=False)
    wv = nc.declare_dram_parameter("wv", [P, KO * DOUT], F16, isOutput=False)
    maskT = nc.declare_dram_parameter("maskT", [P, QG], BF16, isOutput=False)
    ohat = nc.declare_dram_parameter("ohat", [S, DOUT], BF16, isOutput=True)
    l_out = nc.declare_dram_parameter("l", [1, S], F32, isOutput=True)

    xq3 = xqT[:, :].rearrange("(o p) q -> p o q", p=P)
    xk3 = xkT[:, :].rearrange("(o p) s -> p o s", p=P)
    xv3 = xvT[:, :].rearrange("(o p) s -> p o s", p=P)
    # host pre-swizzled: mh4[p, t, k, ii] = Mh.T[k*128+p, t*128+ii] so each
    # dout-slice load is one contiguous 2KB line per partition
    mh4 = mhT[:, :].rearrange("p (t k x) -> p t k x", t=KO, k=KO)
    # wv host layout IS the sbuf layout [p][k][dout]: straight big-line loads
    wv3 = wv[:, :].rearrange("p (k x) -> p k x", k=KO)

    with tile.TileContext(nc) as tc:
        # ---- PE warmup: keep the tensor engine busy through the DMA cold
        # start so the HAM clock ramp (3.4us busy window) completes early.
        with (
            tc.tile_pool(name="warm", bufs=1) as warmpool,
            tc.tile_pool(name="ps_w", bufs=1, space="PSUM") as pswarm,
        ):
            wsc = warmpool.tile([P, 512], F16, name="warm_sc")
            nc.vector.memset(wsc, 0.0)
            wps = pswarm.tile([P, 512], F32, name="warm_ps")
            for i in range(WARM):
                nc.tensor.matmul(
                    wps, lhsT=wsc[:, 0:P], rhs=wsc,
                    start=(i == 0), stop=(i == WARM - 1),
                )

        with tc.tile_pool(name="persist", bufs=1) as pp:
            # all inputs resident; every tile DMA'd exactly once
            mh_sb = pp.tile([P, KO, DIN], F16, name="mh")
            xk_sb = pp.tile([P, KO, KSH], F16, name="xk")
            xv_sb = pp.tile([P, KO, KSH], F16, name="xv")
            xq_sb = pp.tile([P, KO, S], F16, name="xq")
            wv_sb = pp.tile([P, KO, DOUT], F16, name="wv")
            kqt_sb = pp.tile([P, KO, KSH], F16, name="kqt")
            v_sb = pp.tile([P, NT, DOUT], BF16, name="v")
            slab_first = pp.tile([P, NT, QG], BF16, name="expT_first")
            m0_sb = pp.tile([P, QG], BF16, name="mask0")
            zeros_sb = pp.tile([P, QG // 2], BF16, name="zeros")
            ones_sb = pp.tile([P, 1], BF16, name="ones")
            l_sb = pp.tile([1, S], F32, name="l_row")
            nc.vector.memset(zeros_sb, 0.0)
            nc.vector.memset(ones_sb, 1.0)

            # ---- DMA issue order == first-use order.  All targets are
            # fresh resident tiles, so no descriptor ever waits on compute.
            nc.sync.dma_start(out=mh_sb[:, :, 0:P], in_=mh4[:, 0])
            for o in range(0, KO, 2):   # K~ chunk 0 feed, o-pair granularity
                nc.sync.dma_start(
                    out=xk_sb[:, o : o + 2, 0:QG], in_=xk3[:, o : o + 2, 0:QG]
                )
            for s in range(1, KO):      # rest of Mh, slice-wise
                nc.sync.dma_start(
                    out=mh_sb[:, :, s * P : (s + 1) * P], in_=mh4[:, s]
                )
            for o in range(0, KO, 2):   # K~ chunk 1 feed
                nc.sync.dma_start(
                    out=xk_sb[:, o : o + 2, QG:KSH], in_=xk3[:, o : o + 2, QG:KSH]
                )
            g3 = (NG - 1) * QG          # g3 score block inputs
            for o in range(0, KO, 4):
                nc.sync.dma_start(
                    out=xq_sb[:, o : o + 4, g3 : g3 + QG],
                    in_=xq3[:, o : o + 4, g3 : g3 + QG],
                )
            nc.sync.dma_start(out=m0_sb, in_=maskT[:, :])
            for k in range(0, KO, 4):   # V-phase inputs
                nc.sync.dma_start(
                    out=wv_sb[:, k : k + 4, :], in_=wv3[:, k : k + 4, :]
                )
            for o in range(0, KO, 4):
                nc.sync.dma_start(
                    out=xv_sb[:, o : o + 4, :], in_=xv3[:, o : o + 4, :]
                )
            for g in (2, 1, 0):         # remaining query groups, use order
                for o in range(0, KO, 4):
                    nc.sync.dma_start(
                        out=xq_sb[:, o : o + 4, g * QG : (g + 1) * QG],
                        in_=xq3[:, o : o + 4, g * QG : (g + 1) * QG],
                    )

            # ---- Phase K~: K~^T = Mh @ Xk^T  [din, keys]
            with tc.tile_pool(name="ps_k", bufs=4, space="PSUM") as psK:
                for c in range(2):
                    for o in range(KO):
                        ps = psK.tile([P, QG], F32, name="kq_ps")
                        for k in range(KO):
                            nc.tensor.matmul(
                                ps,
                                lhsT=mh_sb[:, k, o * P : (o + 1) * P],
                                rhs=xk_sb[:, k, c * QG : (c + 1) * QG],
                                start=(k == 0),
                                stop=(k == KO - 1),
                            )
                        nc.vector.tensor_copy(
                            kqt_sb[:, o, c * QG : (c + 1) * QG], ps
                        )

            with (
                tc.tile_pool(name="exp_tmp", bufs=2) as epool,
                tc.tile_pool(name="slab", bufs=2) as slabpool,
                tc.tile_pool(name="ao", bufs=3) as aopool,
                tc.tile_pool(name="ps_s", bufs=3, space="PSUM") as psS,
                tc.tile_pool(name="ps_l", bufs=1, space="PSUM") as psL,
            ):
                H = QG // 2

                def score_chunk(slab, g, kt):
                    """Scores+exp for one (group, k-tile) [128, 512] chunk of
                    S^T.  kt == 2g+1's first 256 queries are fully masked for
                    both cores (interleaved-key geometry): zero-fill and
                    compute only the second half.  The causal mask pattern is
                    group-independent, so one resident m0 tile serves every
                    diagonal chunk; masking is a post-exp 0/1 multiply so
                    bf16 rounding never touches raw logits."""
                    q0 = g * QG
                    if kt == 2 * g + 1:
                        ps = psS.tile([P, QG], F32, name="score_ps")
                        ph = ps[:, H:]
                        for io in range(KO):
                            nc.tensor.matmul(
                                ph,
                                lhsT=kqt_sb[:, io, kt * P : (kt + 1) * P],
                                rhs=xq_sb[:, io, q0 + H : q0 + QG],
                                start=(io == 0),
                                stop=(io == KO - 1),
                            )
                        nc.vector.tensor_copy(slab[:, kt, :H], zeros_sb)
                        et = epool.tile([P, QG], BF16, name="exp_tmp")
                        nc.scalar.activation(
                            et[:, :H], ph, mybir.ActivationFunctionType.Exp
                        )
                        nc.vector.tensor_tensor(
                            slab[:, kt, H:], et[:, :H], m0_sb[:, :H],
                            mybir.AluOpType.mult,
                        )
                        return
                    ps = psS.tile([P, QG], F32, name="score_ps")
                    for io in range(KO):
                        nc.tensor.matmul(
                            ps,
                            lhsT=kqt_sb[:, io, kt * P : (kt + 1) * P],
                            rhs=xq_sb[:, io, q0 : q0 + QG],
                            start=(io == 0),
                            stop=(io == KO - 1),
                        )
                    if kt == 2 * g:
                        et = epool.tile([P, QG], BF16, name="exp_tmp")
                        nc.scalar.activation(
                            et, ps, mybir.ActivationFunctionType.Exp
                        )
                        nc.vector.tensor_tensor(
                            slab[:, kt, :], et, m0_sb, mybir.AluOpType.mult
                        )
                    else:
                        nc.scalar.activation(
                            slab[:, kt, :], ps, mybir.ActivationFunctionType.Exp
                        )

                # ---- g3 score block: needs only kqt + xq_g3, runs while the
                # V-phase inputs finish streaming
                for kt in range(NT):
                    score_chunk(slab_first, NG - 1, kt)

                # ---- Phase V: V = Xv @ Wv for this core's key blocks
                with tc.tile_pool(name="ps_v", bufs=4, space="PSUM") as psV:
                    for t in range(NT):
                        for dh in range(2):
                            ps = psV.tile([P, QG], F32, name="v_ps")
                            for k in range(KO):
                                nc.tensor.matmul(
                                    ps,
                                    lhsT=xv_sb[:, k, t * P : (t + 1) * P],
                                    rhs=wv_sb[:, k, dh * QG : (dh + 1) * QG],
                                    start=(k == 0),
                                    stop=(k == KO - 1),
                                )
                            nc.vector.tensor_copy(
                                v_sb[:, t, dh * QG : (dh + 1) * QG], ps
                            )

                # ---- Phase A: causal-skip transposed-softmax attention,
                # largest group first
                psO_cm = tc.tile_pool(name="ps_o", bufs=4, space="PSUM")
                psO = psO_cm.__enter__()
                for g in reversed(range(NG)):
                    lim = min(NT, 2 * g + 2)   # k-tiles actually attended
                    if g == NG - 1:
                        slab = slab_first
                    else:
                        slab = slabpool.tile([P, NT, QG], BF16, name="expT")
                        for kt in range(lim):
                            score_chunk(slab, g, kt)

                    ps_l = psL.tile([1, QG], F32, name="l_ps")
                    for kt in range(lim):
                        nc.tensor.matmul(
                            ps_l,
                            lhsT=ones_sb,
                            rhs=slab[:, kt, :],
                            start=(kt == 0),
                            stop=(kt == lim - 1),
                        )
                    nc.vector.tensor_copy(l_sb[:, g * QG : (g + 1) * QG], ps_l)
                    nc.sync.dma_start(
                        out=l_out[:, g * QG : (g + 1) * QG],
                        in_=l_sb[:, g * QG : (g + 1) * QG],
                    )

                    for t in range(QG // P):
                        # first 256 queries of the group can't see the last
                        # (fully masked) key tile
                        kts = range(lim - 1) if t < 2 else range(lim)
                        o_sb = aopool.tile([P, DOUT], BF16, name="attn_out")
                        q0 = g * QG + t * P
                        for dh in range(2):
                            ps = psO.tile([P, QG], F32, name="out_ps")
                            for kt in kts:
                                nc.tensor.matmul(
                                    ps,
                                    lhsT=slab[:, kt, t * P : (t + 1) * P],
                                    rhs=v_sb[:, kt, dh * QG : (dh + 1) * QG],
                                    start=(kt == kts[0]),
                                    stop=(kt == kts[-1]),
                                )
                            if dh == 0:
                                nc.scalar.copy(o_sb[:, :QG], ps)
                            else:
                                nc.vector.tensor_copy(o_sb[:, QG:], ps)
                        nc.sync.dma_start(
                            out=ohat[q0 : q0 + P, :], in_=o_sb
                        )
                psO_cm.__exit__(None, None, None)
    nc.finalize()
    return nc


def _get_nc():
    if "nc" not in _NC_CACHE:
        _NC_CACHE["nc"] = _build_bass()
    return _NC_CACHE["nc"]


def _key_index(hk):
    """Global key rows owned by core hk: interleaved 128-row blocks."""
    blocks = np.arange(hk, S // P, 2)
    return (blocks[:, None] * P + np.arange(P)[None, :]).reshape(-1)


def _mask_tile(hk):
    """Multiplicative causal mask for the diagonal score chunk: within chunk
    kt == 2g (global key block 4g+hk), key row k masks query column q iff
    k + 128*hk > q; the same inequality covers the kt == 2g+1 half chunk on
    its first 256 columns.  Applied POST-exp as a 0/1 multiply."""
    k_idx = np.arange(P)[:, None] + P * hk
    q_idx = np.arange(QG)[None, :]
    return np.where(k_idx > q_idx, 0.0, 1.0)


def kernel(
    inputs_for_keys,
    inputs_for_values,
    inputs_for_queries,
    WK,
    WV,
    WQ,
    _trace=False,
):
    import ml_dtypes

    F16N = np.float16
    xk = np.asarray(inputs_for_keys, dtype=np.float32)
    xv = np.asarray(inputs_for_values, dtype=np.float32)
    xq = np.asarray(inputs_for_queries, dtype=np.float32)
    # wv host layout == sbuf layout [p][k][dout]
    wv_h = np.ascontiguousarray(
        np.asarray(WV, np.float32).reshape(KO, P, DOUT)
        .transpose(1, 0, 2).reshape(P, -1)
    ).astype(F16N)
    wq = np.asarray(WQ, dtype=np.float32)
    wk = np.asarray(WK, dtype=np.float32)
    # fused score weight: S = Xq (WQ WK^T / sqrt(d)) Xk^T;  mhT = (WQ WK^T).T
    mh_f = ((wk @ wq.T) * np.float32(1.0 / np.sqrt(DOUT))).astype(np.float32)
    # swizzle so each dout-slice is one contiguous line per partition:
    # mh4[p, t*1024 + k*128 + ii] = Mh.T[k*128+p, t*128+ii]
    mhT = np.ascontiguousarray(
        mh_f.reshape(KO, P, KO, P).transpose(1, 2, 0, 3).reshape(P, -1)
    ).astype(F16N)

    masks = {
        hk: _mask_tile(hk).astype(ml_dtypes.bfloat16) for hk in (0, 1)
    }
    kidx = {hk: _key_index(hk) for hk in (0, 1)}
    xqTb = [np.ascontiguousarray(xq[b].T).astype(F16N) for b in range(B)]

    in_maps = []
    for i in range(8):
        b, hk = i // 2, i % 2
        in_maps.append(
            {
                "xqT": xqTb[b],
                "xkT": np.ascontiguousarray(xk[b][kidx[hk]].T).astype(F16N),
                "xvT": np.ascontiguousarray(xv[b][kidx[hk]].T).astype(F16N),
                "mhT": mhT,
                "wv": wv_h,
                "maskT": masks[hk],
            }
        )

    nc = _get_nc()
    res = run_bass_kernel_spmd(nc, in_maps, list(range(8)), trace=_trace)

    out = np.empty((B, S, DOUT), dtype=np.float32)
    for b in range(B):
        r0 = res.results[2 * b]
        r1 = res.results[2 * b + 1]
        den = (
            np.asarray(r0["l"], np.float32) + np.asarray(r1["l"], np.float32)
        ).reshape(S, 1)
        o01 = np.asarray(r0["ohat"], np.float32) + np.asarray(
            r1["ohat"], np.float32
        )
        out[b] = o01 / den
    if _trace:
        return out, res
    return out


# revision 9
# speedup vs baseline: 1.1495x; 1.0241x over previous
"""Trainium2 Bass kernel for single-head causal attention.

Transposed-softmax layout (S^T, no PE transposes, no max-subtraction),
interleaved-key causal skip, host flash-combine, host-side weight
fusion Mh = WK @ WQ^T / sqrt(d), and a fully-resident fp16/bf16 SBUF
plan:

  score path (Xq, Mh, Xk, K~) in fp16  -- 1.0 PE cycle/row like f32r,
      but half the DMA bytes / SBUF footprint and FWL weight loads
  softmax slab, V, ohat in bf16        -- exp() needs bf16's exponent
      range (scores reach ~+/-50, exp up to ~1e22)
  all matmul accumulation in f32 PSUM; host combines
      out = (ohat0 + ohat1) / (l0 + l1) per batch in f32.

Everything is resident in SBUF (~180 KB/partition of 208), so every
input tile is DMA'd exactly once with no write-after-read hazards:
the PE stream never waits on a buffer-reuse semaphore.  Per-core
phases: warmup (bridges DMA cold start + HAM clock ramp) -> K~ =
Mh @ Xk^T -> g3 score block -> V projection -> attention groups
largest-first.
"""

import numpy as np

import concourse.bass as bass
from concourse import bacc
import concourse.mybir as mybir
import concourse.tile as tile
from concourse.bass_utils import run_bass_kernel_spmd

P = 128
B, S, DIN, DOUT = 4, 2048, 1024, 1024
KSH = S // 2        # key rows per core
KO = DIN // P       # 8 contraction sub-tiles
NT = KSH // P       # 8 key tiles per core
QG = 512            # query-group width (psum free dim)
NG = S // QG        # 4 query groups
F32 = mybir.dt.float32
F16 = mybir.dt.float16
BF16 = mybir.dt.bfloat16
WARM = 8            # warmup matmuls (bridge DMA cold start + HAM ramp)

_NC_CACHE = {}


def _build_bass():
    nc = bacc.Bacc()
    xqT = nc.declare_dram_parameter("xqT", [DIN, S], F16, isOutput=False)
    xkT = nc.declare_dram_parameter("xkT", [DIN, KSH], F16, isOutput=False)
    xvT = nc.declare_dram_parameter("xvT", [DIN, KSH], F16, isOutput=False)
    mhT = nc.declare_dram_parameter("mhT", [P, KO * KO * P], F16, isOutput=False)
    wv = nc.declare_dram_parameter("wv", [P, KO * DOUT], F16, isOutput=False)
    maskT = nc.declare_dram_parameter("maskT", [P, QG], BF16, isOutput=False)
    ohat = nc.declare_dram_parameter("ohat", [S, DOUT], BF16, isOutput=True)
    l_out = nc.declare_dram_parameter("l", [1, S], F32, isOutput=True)

    xq3 = xqT[:, :].rearrange("(o p) q -> p o q", p=P)
    xk3 = xkT[:, :].rearrange("(o p) s -> p o s", p=P)
    xv3 = xvT[:, :].rearrange("(o p) s -> p o s", p=P)
    # host pre-swizzled: mh4[p, t, k, ii] = Mh.T[k*128+p, t*128+ii] so each
    # dout-slice load is one contiguous 2KB line per partition
    mh4 = mhT[:, :].rearrange("p (t k x) -> p t k x", t=KO, k=KO)
    # wv host layout IS the sbuf layout [p][k][dout]: straight big-line loads
    wv3 = wv[:, :].rearrange("p (k x) -> p k x", k=KO)

    with tile.TileContext(nc) as tc:
        with tc.tile_pool(name="persist", bufs=1) as pp:
            # ---- PE warmup: keep the tensor engine busy through the DMA
            # cold start so the HAM clock ramp (3.4us busy window) completes
            # early.  The warm tile lives in the persist pool: a scoped pool
            # would be reused by the input tiles below, making their DMAs
            # wait (WAR) for the warmup matmuls.
            with tc.tile_pool(name="ps_w", bufs=1, space="PSUM") as pswarm:
                wsc = pp.tile([P, 512], F16, name="warm_sc")
                nc.vector.memset(wsc, 0.0)
                wps = pswarm.tile([P, 512], F32, name="warm_ps")
                for i in range(WARM):
                    nc.tensor.matmul(
                        wps, lhsT=wsc[:, 0:P], rhs=wsc,
                        start=(i == 0), stop=(i == WARM - 1),
                    )
            # all inputs resident; every tile DMA'd exactly once
            mh_sb = pp.tile([P, KO, DIN], F16, name="mh")
            xk_sb = pp.tile([P, KO, KSH], F16, name="xk")
            xv_sb = pp.tile([P, KO, KSH], F16, name="xv")
            xq_sb = pp.tile([P, KO, S], F16, name="xq")
            wv_sb = pp.tile([P, KO, DOUT], F16, name="wv")
            kqt_sb = pp.tile([P, KO, KSH], F16, name="kqt")
            v_sb = pp.tile([P, NT, DOUT], BF16, name="v")
            slab_first = pp.tile([P, NT, QG], BF16, name="expT_first")
            m0_sb = pp.tile([P, QG], BF16, name="mask0")
            zeros_sb = pp.tile([P, QG // 2], BF16, name="zeros")
            ones_sb = pp.tile([P, 1], BF16, name="ones")
            l_sb = pp.tile([1, S], F32, name="l_row")
            nc.vector.memset(zeros_sb, 0.0)
            nc.vector.memset(ones_sb, 1.0)

            # ---- DMA issue order == first-use order.  All targets are
            # fresh resident tiles, so no descriptor ever waits on compute.
            nc.sync.dma_start(out=mh_sb[:, :, 0:P], in_=mh4[:, 0])
            for o in range(0, KO, 2):   # K~ chunk 0 feed, o-pair granularity
                nc.sync.dma_start(
                    out=xk_sb[:, o : o + 2, 0:QG], in_=xk3[:, o : o + 2, 0:QG]
                )
            for s in range(1, KO):      # rest of Mh, slice-wise
                nc.sync.dma_start(
                    out=mh_sb[:, :, s * P : (s + 1) * P], in_=mh4[:, s]
                )
            for o in range(0, KO, 2):   # K~ chunk 1 feed
                nc.sync.dma_start(
                    out=xk_sb[:, o : o + 2, QG:KSH], in_=xk3[:, o : o + 2, QG:KSH]
                )
            g3 = (NG - 1) * QG          # g3 score block inputs
            for o in range(0, KO, 4):
                nc.sync.dma_start(
                    out=xq_sb[:, o : o + 4, g3 : g3 + QG],
                    in_=xq3[:, o : o + 4, g3 : g3 + QG],
                )
            nc.sync.dma_start(out=m0_sb, in_=maskT[:, :])
            for k in range(0, KO, 4):   # V-phase inputs
                nc.sync.dma_start(
                    out=wv_sb[:, k : k + 4, :], in_=wv3[:, k : k + 4, :]
                )
            for o in range(0, KO, 4):
                nc.sync.dma_start(
                    out=xv_sb[:, o : o + 4, :], in_=xv3[:, o : o + 4, :]
                )
            for g in (2, 1, 0):         # remaining query groups, use order
                for o in range(0, KO, 4):
                    nc.sync.dma_start(
                        out=xq_sb[:, o : o + 4, g * QG : (g + 1) * QG],
                        in_=xq3[:, o : o + 4, g * QG : (g + 1) * QG],
                    )

            # ---- Phase K~: K~^T = Mh @ Xk^T  [din, keys]
            with tc.tile_pool(name="ps_k", bufs=4, space="PSUM") as psK:
                for c in range(2):
                    for o in range(KO):
                        ps = psK.tile([P, QG], F32, name="kq_ps")
                        for k in range(KO):
                            nc.tensor.matmul(
                                ps,
                                lhsT=mh_sb[:, k, o * P : (o + 1) * P],
                                rhs=xk_sb[:, k, c * QG : (c + 1) * QG],
                                start=(k == 0),
                                stop=(k == KO - 1),
                            )
                        nc.vector.tensor_copy(
                            kqt_sb[:, o, c * QG : (c + 1) * QG], ps
                        )

            with (
                tc.tile_pool(name="exp_tmp", bufs=2) as epool,
                tc.tile_pool(name="slab", bufs=2) as slabpool,
                tc.tile_pool(name="ao", bufs=3) as aopool,
                tc.tile_pool(name="ps_s", bufs=3, space="PSUM") as psS,
                tc.tile_pool(name="ps_l", bufs=1, space="PSUM") as psL,
            ):
                H = QG // 2

                def score_chunk(slab, g, kt):
                    """Scores+exp for one (group, k-tile) [128, 512] chunk of
                    S^T.  kt == 2g+1's first 256 queries are fully masked for
                    both cores (interleaved-key geometry): zero-fill and
                    compute only the second half.  The causal mask pattern is
                    group-independent, so one resident m0 tile serves every
                    diagonal chunk; masking is a post-exp 0/1 multiply so
                    bf16 rounding never touches raw logits."""
                    q0 = g * QG
                    if kt == 2 * g + 1:
                        ps = psS.tile([P, QG], F32, name="score_ps")
                        ph = ps[:, H:]
                        for io in range(KO):
                            nc.tensor.matmul(
                                ph,
                                lhsT=kqt_sb[:, io, kt * P : (kt + 1) * P],
                                rhs=xq_sb[:, io, q0 + H : q0 + QG],
                                start=(io == 0),
                                stop=(io == KO - 1),
                            )
                        nc.vector.tensor_copy(slab[:, kt, :H], zeros_sb)
                        et = epool.tile([P, QG], BF16, name="exp_tmp")
                        nc.scalar.activation(
                            et[:, :H], ph, mybir.ActivationFunctionType.Exp
                        )
                        nc.vector.tensor_tensor(
                            slab[:, kt, H:], et[:, :H], m0_sb[:, :H],
                            mybir.AluOpType.mult,
                        )
                        return
                    ps = psS.tile([P, QG], F32, name="score_ps")
                    for io in range(KO):
                        nc.tensor.matmul(
                            ps,
                            lhsT=kqt_sb[:, io, kt * P : (kt + 1) * P],
                            rhs=xq_sb[:, io, q0 : q0 + QG],
                            start=(io == 0),
                            stop=(io == KO - 1),
                        )
                    if kt == 2 * g:
                        et = epool.tile([P, QG], BF16, name="exp_tmp")
                        nc.scalar.activation(
                            et, ps, mybir.ActivationFunctionType.Exp
                        )
                        nc.vector.tensor_tensor(
                            slab[:, kt, :], et, m0_sb, mybir.AluOpType.mult
                        )
                    else:
                        nc.scalar.activation(
                            slab[:, kt, :], ps, mybir.ActivationFunctionType.Exp
                        )

                # ---- g3 score block: needs only kqt + xq_g3, runs while the
                # V-phase inputs finish streaming
                for kt in range(NT):
                    score_chunk(slab_first, NG - 1, kt)

                # ---- Phase V: V = Xv @ Wv for this core's key blocks
                with tc.tile_pool(name="ps_v", bufs=4, space="PSUM") as psV:
                    for t in range(NT):
                        for dh in range(2):
                            ps = psV.tile([P, QG], F32, name="v_ps")
                            for k in range(KO):
                                nc.tensor.matmul(
                                    ps,
                                    lhsT=xv_sb[:, k, t * P : (t + 1) * P],
                                    rhs=wv_sb[:, k, dh * QG : (dh + 1) * QG],
                                    start=(k == 0),
                                    stop=(k == KO - 1),
                                )
                            nc.vector.tensor_copy(
                                v_sb[:, t, dh * QG : (dh + 1) * QG], ps
                            )

                # ---- Phase A: causal-skip transposed-softmax attention,
                # largest group first
                psO_cm = tc.tile_pool(name="ps_o", bufs=4, space="PSUM")
                psO = psO_cm.__enter__()
                for g in reversed(range(NG)):
                    lim = min(NT, 2 * g + 2)   # k-tiles actually attended
                    if g == NG - 1:
                        slab = slab_first
                    else:
                        slab = slabpool.tile([P, NT, QG], BF16, name="expT")
                        for kt in range(lim):
                            score_chunk(slab, g, kt)

                    ps_l = psL.tile([1, QG], F32, name="l_ps")
                    for kt in range(lim):
                        nc.tensor.matmul(
                            ps_l,
                            lhsT=ones_sb,
                            rhs=slab[:, kt, :],
                            start=(kt == 0),
                            stop=(kt == lim - 1),
                        )
                    nc.vector.tensor_copy(l_sb[:, g * QG : (g + 1) * QG], ps_l)
                    nc.sync.dma_start(
                        out=l_out[:, g * QG : (g + 1) * QG],
                        in_=l_sb[:, g * QG : (g + 1) * QG],
                    )

                    for t in range(QG // P):
                        # first 256 queries of the group can't see the last
                        # (fully masked) key tile
                        kts = range(lim - 1) if t < 2 else range(lim)
                        o_sb = aopool.tile([P, DOUT], BF16, name="attn_out")
                        q0 = g * QG + t * P
                        for dh in range(2):
                            ps = psO.tile([P, QG], F32, name="out_ps")
                            for kt in kts:
                                nc.tensor.matmul(
                                    ps,
                                    lhsT=slab[:, kt, t * P : (t + 1) * P],
                                    rhs=v_sb[:, kt, dh * QG : (dh + 1) * QG],
                                    start=(kt == kts[0]),
                                    stop=(kt == kts[-1]),
                                )
                            if dh == 0:
                                nc.scalar.copy(o_sb[:, :QG], ps)
                            else:
                                nc.vector.tensor_copy(o_sb[:, QG:], ps)
                            # per-dh stores so the last transfer pipelines
                            # with the last psum->sbuf copy
                            nc.sync.dma_start(
                                out=ohat[q0 : q0 + P, dh * QG : (dh + 1) * QG],
                                in_=o_sb[:, dh * QG : (dh + 1) * QG],
                            )
                psO_cm.__exit__(None, None, None)
    nc.finalize()
    return nc


def _get_nc():
    if "nc" not in _NC_CACHE:
        _NC_CACHE["nc"] = _build_bass()
    return _NC_CACHE["nc"]


def _key_index(hk):
    """Global key rows owned by core hk: interleaved 128-row blocks."""
    blocks = np.arange(hk, S // P, 2)
    return (blocks[:, None] * P + np.arange(P)[None, :]).reshape(-1)


def _mask_tile(hk):
    """Multiplicative causal mask for the diagonal score chunk: within chunk
    kt == 2g (global key block 4g+hk), key row k masks query column q iff
    k + 128*hk > q; the same inequality covers the kt == 2g+1 half chunk on
    its first 256 columns.  Applied POST-exp as a 0/1 multiply."""
    k_idx = np.arange(P)[:, None] + P * hk
    q_idx = np.arange(QG)[None, :]
    return np.where(k_idx > q_idx, 0.0, 1.0)


def kernel(
    inputs_for_keys,
    inputs_for_values,
    inputs_for_queries,
    WK,
    WV,
    WQ,
    _trace=False,
):
    import ml_dtypes

    F16N = np.float16
    xk = np.asarray(inputs_for_keys, dtype=np.float32)
    xv = np.asarray(inputs_for_values, dtype=np.float32)
    xq = np.asarray(inputs_for_queries, dtype=np.float32)
    # wv host layout == sbuf layout [p][k][dout]
    wv_h = np.ascontiguousarray(
        np.asarray(WV, np.float32).reshape(KO, P, DOUT)
        .transpose(1, 0, 2).reshape(P, -1)
    ).astype(F16N)
    wq = np.asarray(WQ, dtype=np.float32)
    wk = np.asarray(WK, dtype=np.float32)
    # fused score weight: S = Xq (WQ WK^T / sqrt(d)) Xk^T;  mhT = (WQ WK^T).T
    mh_f = ((wk @ wq.T) * np.float32(1.0 / np.sqrt(DOUT))).astype(np.float32)
    # swizzle so each dout-slice is one contiguous line per partition:
    # mh4[p, t*1024 + k*128 + ii] = Mh.T[k*128+p, t*128+ii]
    mhT = np.ascontiguousarray(
        mh_f.reshape(KO, P, KO, P).transpose(1, 2, 0, 3).reshape(P, -1)
    ).astype(F16N)

    masks = {
        hk: _mask_tile(hk).astype(ml_dtypes.bfloat16) for hk in (0, 1)
    }
    kidx = {hk: _key_index(hk) for hk in (0, 1)}
    xqTb = [np.ascontiguousarray(xq[b].T).astype(F16N) for b in range(B)]

    in_maps = []
    for i in range(8):
        b, hk = i // 2, i % 2
        in_maps.append(
            {
                "xqT": xqTb[b],
                "xkT": np.ascontiguousarray(xk[b][kidx[hk]].T).astype(F16N),
                "xvT": np.ascontiguousarray(xv[b][kidx[hk]].T).astype(F16N),
                "mhT": mhT,
                "wv": wv_h,
                "maskT": masks[hk],
            }
        )

    nc = _get_nc()
    res = run_bass_kernel_spmd(nc, in_maps, list(range(8)), trace=_trace)

    out = np.empty((B, S, DOUT), dtype=np.float32)
    for b in range(B):
        r0 = res.results[2 * b]
        r1 = res.results[2 * b + 1]
        den = (
            np.asarray(r0["l"], np.float32) + np.asarray(r1["l"], np.float32)
        ).reshape(S, 1)
        o01 = np.asarray(r0["ohat"], np.float32) + np.asarray(
            r1["ohat"], np.float32
        )
        out[b] = o01 / den
    if _trace:
        return out, res
    return out
